# revision 34
# baseline (speedup 1.0000x reference)
"""Multi-head attention (B=2, S=2048, D=1024, H=16) on 8 Trainium2 cores.

Sharding: core = 4*b + g  (b = batch 0..1, g = head-group 0..3, 4 heads each).
Each core computes, for its batch b and head-group g (256 of the 1024 dims):
  QT/KT = (x @ W^T)^T  in [d, s] layout   (d on partitions)
  V     = x @ W^T      in [s, d] layout   (s on partitions)
  ST    = scores^T     in [k, q] layout   (k on partitions)  -> exp on ACT
  U     = V^T @ P^T    in [d, q] layout + per-head denominators Z via ones-matmul
  UN    = U / Z        (PE-broadcast reciprocal, DVE multiply)
  Ypart = UN^T @ WoT   in [q, e] layout   (partial over this group's 256 dims)
Host sums the 4 per-group partials per batch and adds b_o.

All matmuls run in float32r (TF32-like, 1 cycle/row at N>=256).
"""

import os
from contextlib import ExitStack

import numpy as np

import concourse.bass as bass
import concourse.tile as tile
from concourse import bacc, mybir
from concourse.tile import add_dep_helper

B, S, D = 2, 2048, 1024
H, DH = 16, 64
NCORES = 8
NG = 4                  # head-group shards
DG = D // NG            # 256 dims per head-group (4 heads)
P = 128
QC = 512                # q-chunk width
NQC = S // QC           # 4
NKT = S // P            # 16 k-tiles of 128
CD = D // P             # 8 contraction tiles for the projections
F32 = mybir.dt.float32
F32R = mybir.dt.float32r
AF = mybir.ActivationFunctionType
SCALE = 1.0 / float(np.sqrt(D))






def _body(ctx: ExitStack, tc: "tile.TileContext", io: dict):
    nc = tc.nc
    # f32r is bit-identical to f32; the "low precision" is the PE's TF32-style
    # rounding, which we accept deliberately for 4x matmul throughput.
    ctx.enter_context(nc.allow_low_precision(reason="f32r matmul pipeline"))
    sb = ctx.enter_context(tc.tile_pool(name="sb", bufs=1))

    # --- constants (memset can't target f32r; DMA from a ones input) -------
    ones_col = sb.tile([1, P], F32R, tag="ones_col", bufs=1, name="ones_col")
    nc.sync.dma_start(ones_col[:], io["ones"][None, :])

    # --- biases ------------------------------------------------------------
    bq = sb.tile([P, 2], F32, tag="bq", bufs=1, name="bq")
    nc.sync.dma_start(bq[:], io["bq"].rearrange("(t p) -> p t", p=P))
    bk = sb.tile([P, 2], F32, tag="bk", bufs=1, name="bk")
    nc.sync.dma_start(bk[:], io["bk"].rearrange("(t p) -> p t", p=P))
    bv_row = sb.tile([1, DG], F32R, tag="bv", bufs=1, name="bv_row")
    nc.sync.dma_start(bv_row[:], io["bv"][None, :])

    # --- phase 1: projections (own PSUM pool, 8 banks) ---------------------
    QT, KT = {}, {}
    V = {}
    with tc.tile_pool(name="ps_proj", bufs=1, space="PSUM") as ps1:
        for nm, xkey, wkey, bias, outmap in (
            ("q", "xq", "wq", bq, QT),
            ("k", "xk", "wk", bk, KT),
        ):
            w = sb.tile([P, CD, DG], F32R, tag="w", bufs=2, name=f"w{nm}")
            nc.sync.dma_start(w[:], io[wkey].rearrange("(c p) d -> p c d", p=P))
            psg = {}
            for d in range(2):
                for sc in range(NQC):
                    psg[d, sc] = ps1.tile(
                        [P, QC], F32, tag="proj", bufs=8, name=f"ps_{nm}{d}{sc}"
                    )
            for c in range(CD):
                xt = sb.tile([P, S], F32R, tag="x", bufs=4, name=f"x{nm}{c}")
                nc.sync.dma_start(xt[:], io[xkey][c * P : (c + 1) * P, :])
                for d in range(2):
                    for sc in range(NQC):
                        nc.tensor.matmul(
                            psg[d, sc][:],
                            (w[:, c, d * P : (d + 1) * P]),
                            (xt[:, sc * QC : (sc + 1) * QC]),
                            start=(c == 0),
                            stop=(c == CD - 1),
                        )
            for d in range(2):
                for sc in range(NQC):
                    t = sb.tile([P, QC], F32R, tag=f"{nm}t", bufs=8, name=f"{nm}T{d}{sc}")
                    nc.vector.tensor_scalar_add(t[:], psg[d, sc][:], bias[:, d : d + 1])
                    outmap[d, sc] = t

        # V projection: V [2048, 256] as 16 tiles of [128, 256]; bias b_v is
        # folded in by seeding each PSUM accumulation with ones_col^T @ bv_row.
        wv = sb.tile([P, CD, DG], F32R, tag="w", bufs=2, name="wv")
        nc.sync.dma_start(wv[:], io["wv"].rearrange("(c p) d -> p c d", p=P))
        psv = {
            sp: ps1.tile([P, 2, DG], F32, tag="proj", bufs=8, name=f"psv{sp}")
            for sp in range(8)
        }
        for sp in range(8):
            seed = None
            for j in range(2):
                mm = nc.tensor.matmul(
                    psv[sp][:, j, :],
                    (ones_col[:, 0:P]),
                    (bv_row[:]),
                    start=(j == 0),
                    stop=False,
                )
                # start=True must execute before any other matmul in the bank;
                # disjoint-slice writes carry no natural dep, so add one.
                if j == 0:
                    seed = mm
                else:
                    add_dep_helper(mm.ins, seed.ins, reason="psum group order")
        last_j0 = {}
        for c in range(CD):
            xt = sb.tile([P, S], F32R, tag="x", bufs=4, name=f"xv{c}")
            nc.sync.dma_start(xt[:], io["xv"][c * P : (c + 1) * P, :])
            for sp in range(8):
                for j in range(2):
                    st_i = sp * 2 + j
                    mm = nc.tensor.matmul(
                        psv[sp][:, j, :],
                        (xt[:, st_i * P : (st_i + 1) * P]),
                        (wv[:, c, :]),
                        start=False,
                        stop=(c == CD - 1 and j == 1),
                    )
                    if j == 0:
                        last_j0[sp] = mm
                    elif c == CD - 1:
                        # stop=True closes the whole bank's group; it must run
                        # after the other slice's last matmul.
                        add_dep_helper(mm.ins, last_j0[sp].ins, reason="psv stop order")
        # V_aug tiles [128, 4, 65]: per head 64 V columns + a ones column that
        # accumulates the softmax denominator into row 64 of U_h.
        ones4 = sb.tile([P, 4], F32R, tag="ones4", bufs=1, name="ones4")
        nc.sync.dma_start(ones4[:], io["ones4"][:])
        for sp in range(8):
            for j in range(2):
                vt = sb.tile([P, 4, DH + 1], F32R, tag="v", bufs=16, name=f"V{sp}_{j}")
                nc.vector.tensor_copy(
                    vt[:, :, 0:DH],
                    psv[sp][:, j, :].rearrange("p (g d) -> p g d", g=4),
                )
                nc.vector.tensor_copy(vt[:, :, DH : DH + 1], ones4[:, :, None])
                V[sp * 2 + j] = vt

    # --- output projection weights -----------------------------------------
    woT = []
    for pr in range(2):
        t = sb.tile([P, D], F32R, tag="wo", bufs=2, name=f"woT{pr}")
        nc.sync.dma_start(t[:], io["wo"][pr * P : (pr + 1) * P, :])
        woT.append(t)

    # --- attention, per q-chunk --------------------------------------------
    # Per-head PSUM accumulators U_h [65, 512]: rows 0..63 are sum_k P*V, row
    # 64 is the softmax denominator (from V_aug's ones column). All matmul
    # outputs start at partition 0 (col-offset tile_position fails walrus
    # codegen in this toolchain).
    ps2 = ctx.enter_context(tc.tile_pool(name="ps_attn", bufs=1, space="PSUM"))
    UN = {}
    for qc in range(NQC):
        U = {
            h: ps2.tile([P, QC], F32, tag="u", bufs=4, name=f"U{qc}_{h}")
            for h in range(4)
        }
        for kg in range(NKT // 2):
            for h in range(4):
                pr, lo = h // 2, (h % 2) * 64
                st2 = ps2.tile([P, 2, QC], F32, tag="st", bufs=2, name=f"st{qc}_{kg}_{h}")
                for kk in range(2):
                    k_tile = kg * 2 + kk
                    sc, off = divmod(k_tile, 4)
                    nc.tensor.matmul(
                        st2[:, kk, :],
                        (KT[pr, sc][lo : lo + 64, off * P : (off + 1) * P]),
                        (QT[pr, qc][lo : lo + 64, :]),
                        start=True,
                        stop=True,
                        tile_position=(lo, 0),
                    )
                pt2 = sb.tile([P, 2, QC], F32R, tag="pt", bufs=3, name=f"pt{qc}_{kg}_{h}")
                nc.scalar.activation(pt2[:], st2[:], AF.Exp, scale=SCALE)
                for kk in range(2):
                    k_tile = kg * 2 + kk
                    nc.tensor.matmul(
                        U[h][0:65, :],
                        (V[k_tile][:, h, :]),
                        (pt2[:, kk, :]),
                        start=(kg == 0 and kk == 0),
                        stop=(kg == NKT // 2 - 1 and kk == 1),
                    )

        # normalize: UN_pair rows = U_h[0:64] * (1/Z_h); odd head's rows are
        # DMA-shifted into partitions 64..127 of the pair tile.
        for pr in range(2):
            UN[qc, pr] = sb.tile([P, QC], F32R, tag="un", bufs=8, name=f"UN{qc}_{pr}")
        for h in range(4):
            pr, off = h // 2, (h % 2) * 64
            zs = sb.tile([65, QC], F32, tag="zs", bufs=2, name=f"zs{qc}_{h}")
            nc.vector.tensor_copy(zs[64:65, :], U[h][64:65, :])
            z0 = sb.tile([1, QC], F32, tag="z0", bufs=2, name=f"z0_{qc}_{h}")
            nc.sync.dma_start(z0[:], zs[64:65, :])
            rz = sb.tile([1, QC], F32, tag="rz", bufs=2, name=f"rz{qc}_{h}")
            nc.vector.reciprocal(rz[:], z0[:])
            rb = sb.tile([64, QC], F32, tag="rb", bufs=2, name=f"rb{qc}_{h}")
            nc.gpsimd.partition_broadcast(rb[:], rz[:], channels=64)
            if off == 0:
                nc.vector.tensor_mul(UN[qc, pr][0:64, :], U[h][0:64, :], rb[:])
            else:
                tmp = sb.tile([64, QC], F32R, tag="untmp", bufs=2, name=f"untmp{qc}_{h}")
                nc.vector.tensor_mul(tmp[:], U[h][0:64, :], rb[:])
                nc.sync.dma_start(UN[qc, pr][64:128, :], tmp[:])

    # --- output projection (after attention; PSUM banks are free then) -----
    for qt in range(S // P):
        qc, qi = divmod(qt, 4)
        ysb = sb.tile([P, D], F32, tag="y", bufs=2, name=f"Y{qt}")
        for ec in range(2):
            yps = ps2.tile([P, QC], F32, tag="st", bufs=2, name=f"yp{qt}_{ec}")
            for pr in range(2):
                nc.tensor.matmul(
                    yps[:],
                    (UN[qc, pr][:, qi * P : (qi + 1) * P]),
                    (woT[pr][:, ec * QC : (ec + 1) * QC]),
                    start=(pr == 0),
                    stop=(pr == 1),
                )
            nc.vector.tensor_copy(ysb[:, ec * QC : (ec + 1) * QC], yps[:])
        nc.sync.dma_start(io["y"][qt * P : (qt + 1) * P, :], ysb[:])


def build_program():
    nc = bacc.Bacc(
        "TRN2", target_bir_lowering=False, debug=False, num_devices=NCORES
    )
    io = {
        "xq": nc.dram_tensor("xq", [D, S], F32R, kind="ExternalInput").ap(),
        "xk": nc.dram_tensor("xk", [D, S], F32R, kind="ExternalInput").ap(),
        "xv": nc.dram_tensor("xv", [D, S], F32R, kind="ExternalInput").ap(),
        "wq": nc.dram_tensor("wq", [D, DG], F32R, kind="ExternalInput").ap(),
        "wk": nc.dram_tensor("wk", [D, DG], F32R, kind="ExternalInput").ap(),
        "wv": nc.dram_tensor("wv", [D, DG], F32R, kind="ExternalInput").ap(),
        "wo": nc.dram_tensor("wo", [DG, D], F32R, kind="ExternalInput").ap(),
        "bq": nc.dram_tensor("bq", [DG], F32, kind="ExternalInput").ap(),
        "bk": nc.dram_tensor("bk", [DG], F32, kind="ExternalInput").ap(),
        "bv": nc.dram_tensor("bv", [DG], F32R, kind="ExternalInput").ap(),
        "ones": nc.dram_tensor("ones", [P], F32R, kind="ExternalInput").ap(),
        "ones4": nc.dram_tensor("ones4", [P, 4], F32R, kind="ExternalInput").ap(),
        "y": nc.dram_tensor("y", [S, D], F32, kind="ExternalOutput").ap(),
    }
    with tile.TileContext(nc) as tc:
        with ExitStack() as ctx:
            _body(ctx, tc, io)
    nc.compile()
    return nc


_CACHE = {}


def _get_program():
    if "nc" not in _CACHE:
        _CACHE["nc"] = build_program()
    return _CACHE["nc"]


def make_in_maps(inputs):
    q = np.asarray(inputs["query"], np.float32)
    k = np.asarray(inputs["key"], np.float32)
    v = np.asarray(inputs["value"], np.float32)
    W_q = np.asarray(inputs["W_q"], np.float32)
    W_k = np.asarray(inputs["W_k"], np.float32)
    W_v = np.asarray(inputs["W_v"], np.float32)
    W_o = np.asarray(inputs["W_o"], np.float32)
    b_q = np.asarray(inputs["b_q"], np.float32)
    b_k = np.asarray(inputs["b_k"], np.float32)
    b_v = np.asarray(inputs["b_v"], np.float32)

    xT = [
        [np.ascontiguousarray(x[b].T) for b in range(B)] for x in (q, k, v)
    ]
    in_maps = []
    for core in range(NCORES):
        b, g = divmod(core, NG)
        sl = slice(g * DG, (g + 1) * DG)
        in_maps.append(
            {
                "xq": xT[0][b],
                "xk": xT[1][b],
                "xv": xT[2][b],
                "wq": np.ascontiguousarray(W_q[sl, :].T),
                "wk": np.ascontiguousarray(W_k[sl, :].T),
                "wv": np.ascontiguousarray(W_v[sl, :].T),
                "wo": np.ascontiguousarray(W_o[:, sl].T),
                "bq": np.ascontiguousarray(b_q[sl]),
                "bk": np.ascontiguousarray(b_k[sl]),
                "bv": np.ascontiguousarray(b_v[sl]),
                "ones": np.ones(P, np.float32),
                "ones4": np.ones((P, 4), np.float32),
            }
        )
    return in_maps


def kernel(**inputs):
    from concourse.bass_utils import run_bass_kernel_spmd

    nc = _get_program()
    in_maps = make_in_maps(inputs)
    trace = bool(int(os.environ.get("MHA_TRACE", "0")))
    res = run_bass_kernel_spmd(nc, in_maps, list(range(NCORES)), trace=trace)
    _CACHE["last_results"] = res

    b_o = np.asarray(inputs["b_o"], np.float32)
    out = np.zeros((B, S, D), np.float32)
    for core in range(NCORES):
        b = core // NG
        out[b] += res.results[core]["y"]
    out += b_o[None, None, :]
    return out


# revision 43
# speedup vs baseline: 1.3053x; 1.3053x over previous
"""Multi-head attention (B=2, S=2048, D=1024, H=16) on 8 Trainium2 cores.

Sharding: core = 4*b + g  (b = batch 0..1, g = head-group 0..3, 4 heads each).
Each core computes, for its batch b and head-group g (256 of the 1024 dims):
  QT/KT = (x @ W^T)^T  in [d, s] layout   (d on partitions)
  V     = x @ W^T      in [s, d] layout   (s on partitions)
  ST    = scores^T     in [k, q] layout   (k on partitions)  -> exp on ACT
  U     = V^T @ P^T    in [d, q] layout + per-head denominators Z via ones-matmul
  UN    = U / Z        (PE-broadcast reciprocal, DVE multiply)
  Ypart = UN^T @ WoT   in [q, e] layout   (partial over this group's 256 dims)
Host sums the 4 per-group partials per batch and adds b_o.

All matmuls run in bfloat16 (1 cycle/row, FWL weight loads).
"""

import os
from contextlib import ExitStack

import ml_dtypes
import numpy as np

import concourse.bass as bass
import concourse.tile as tile
from concourse import bacc, mybir
from concourse.tile import add_dep_helper

B, S, D = 2, 2048, 1024
H, DH = 16, 64
NCORES = 8
NG = 4                  # head-group shards
DG = D // NG            # 256 dims per head-group (4 heads)
P = 128
QC = 512                # q-chunk width
NQC = S // QC           # 4
NKT = S // P            # 16 k-tiles of 128
CD = D // P             # 8 contraction tiles for the projections
F32 = mybir.dt.float32
BF16 = mybir.dt.bfloat16
AF = mybir.ActivationFunctionType
SCALE = 1.0 / float(np.sqrt(D))






def _body(ctx: ExitStack, tc: "tile.TileContext", io: dict):
    nc = tc.nc
    # bf16 operands feed the PE at full rate (1 cycle/row + fast weight load);
    # accumulation stays fp32 in PSUM.
    ctx.enter_context(nc.allow_low_precision(reason="bf16 matmul pipeline"))
    sb = ctx.enter_context(tc.tile_pool(name="sb", bufs=1))

    # --- constants (memset can't target f32r; DMA from a ones input) -------
    ones_col = sb.tile([1, P], BF16, tag="ones_col", bufs=1, name="ones_col")
    nc.sync.dma_start(ones_col[:], io["ones"][None, :])

    # --- biases ------------------------------------------------------------
    bq = sb.tile([P, 2], F32, tag="bq", bufs=1, name="bq")
    nc.sync.dma_start(bq[:], io["bq"].rearrange("(t p) -> p t", p=P))
    bk = sb.tile([P, 2], F32, tag="bk", bufs=1, name="bk")
    nc.sync.dma_start(bk[:], io["bk"].rearrange("(t p) -> p t", p=P))
    bv_row = sb.tile([1, DG], BF16, tag="bv", bufs=1, name="bv_row")
    nc.sync.dma_start(bv_row[:], io["bv"][None, :])

    # --- phase 1: projections (own PSUM pool, 8 banks) ---------------------
    # Order Q -> V -> K: attention's PSUM pool allocation waits for this
    # pool's release, so the last projection should be the one attention
    # needs first (K chunk 0 for the first scores).
    QT, KT = {}, {}
    V = {}

    def qk_proj(ps1, nm, xkey, wkey, bias, outmap):
        w = sb.tile([P, CD, DG], BF16, tag="w", bufs=2, name=f"w{nm}")
        nc.sync.dma_start(w[:], io[wkey].rearrange("(c p) d -> p c d", p=P))
        psg = {}
        for d in range(2):
            for sc in range(NQC):
                psg[d, sc] = ps1.tile(
                    [P, QC], F32, tag="proj", bufs=8, name=f"ps_{nm}{d}{sc}"
                )
        for c in range(CD):
            xt = sb.tile([P, S], BF16, tag="x", bufs=4, name=f"x{nm}{c}")
            nc.sync.dma_start(xt[:], io[xkey][c * P : (c + 1) * P, :])
            for d in range(2):
                for sc in range(NQC):
                    nc.tensor.matmul(
                        psg[d, sc][:],
                        (w[:, c, d * P : (d + 1) * P]),
                        (xt[:, sc * QC : (sc + 1) * QC]),
                        start=(c == 0),
                        stop=(c == CD - 1),
                    )
        for d in range(2):
            for sc in range(NQC):
                t = sb.tile([P, QC], BF16, tag=f"{nm}t", bufs=8, name=f"{nm}T{d}{sc}")
                nc.vector.tensor_scalar_add(t[:], psg[d, sc][:], bias[:, d : d + 1])
                outmap[d, sc] = t

    with tc.tile_pool(name="ps_proj", bufs=1, space="PSUM") as ps1:
        qk_proj(ps1, "q", "xq", "wq", bq, QT)

        # V projection: V [2048, 256] as 16 tiles of [128, 256]; bias b_v is
        # folded in by seeding each PSUM accumulation with ones_col^T @ bv_row.
        wv = sb.tile([P, CD, DG], BF16, tag="w", bufs=2, name="wv")
        nc.sync.dma_start(wv[:], io["wv"].rearrange("(c p) d -> p c d", p=P))
        psv = {
            sp: ps1.tile([P, 2, DG], F32, tag="proj", bufs=8, name=f"psv{sp}")
            for sp in range(8)
        }
        for sp in range(8):
            seed = None
            for j in range(2):
                mm = nc.tensor.matmul(
                    psv[sp][:, j, :],
                    (ones_col[:, 0:P]),
                    (bv_row[:]),
                    start=(j == 0),
                    stop=False,
                )
                # start=True must execute before any other matmul in the bank;
                # disjoint-slice writes carry no natural dep, so add one.
                if j == 0:
                    seed = mm
                else:
                    add_dep_helper(mm.ins, seed.ins, reason="psum group order")
        last_j0 = {}
        for c in range(CD):
            xt = sb.tile([P, S], BF16, tag="x", bufs=4, name=f"xv{c}")
            nc.sync.dma_start(xt[:], io["xv"][c * P : (c + 1) * P, :])
            for sp in range(8):
                for j in range(2):
                    st_i = sp * 2 + j
                    mm = nc.tensor.matmul(
                        psv[sp][:, j, :],
                        (xt[:, st_i * P : (st_i + 1) * P]),
                        (wv[:, c, :]),
                        start=False,
                        stop=(c == CD - 1 and j == 1),
                    )
                    if j == 0:
                        last_j0[sp] = mm
                    elif c == CD - 1:
                        # stop=True closes the whole bank's group; it must run
                        # after the other slice's last matmul.
                        add_dep_helper(mm.ins, last_j0[sp].ins, reason="psv stop order")
        # V_aug tiles [128, 4, 65]: per head 64 V columns + a ones column that
        # accumulates the softmax denominator into row 64 of U_h.
        ones4 = sb.tile([P, 4], BF16, tag="ones4", bufs=1, name="ones4")
        nc.sync.dma_start(ones4[:], io["ones4"][:])
        for sp in range(8):
            for j in range(2):
                vt = sb.tile([P, 4, DH + 1], BF16, tag="v", bufs=16, name=f"V{sp}_{j}")
                nc.vector.tensor_copy(
                    vt[:, :, 0:DH],
                    psv[sp][:, j, :].rearrange("p (g d) -> p g d", g=4),
                )
                nc.vector.tensor_copy(vt[:, :, DH : DH + 1], ones4[:, :, None])
                V[sp * 2 + j] = vt

        qk_proj(ps1, "k", "xk", "wk", bk, KT)

    # --- output projection weights -----------------------------------------
    woT = []
    for pr in range(2):
        t = sb.tile([P, D], BF16, tag="wo", bufs=2, name=f"woT{pr}")
        nc.sync.dma_start(t[:], io["wo"][pr * P : (pr + 1) * P, :])
        woT.append(t)

    # --- attention, per q-chunk --------------------------------------------
    # Per-head PSUM accumulators U_h [65, 512]: rows 0..63 are sum_k P*V, row
    # 64 is the softmax denominator (from V_aug's ones column). All matmul
    # outputs start at partition 0 (col-offset tile_position fails walrus
    # codegen in this toolchain).
    ps2 = ctx.enter_context(tc.tile_pool(name="ps_attn", bufs=1, space="PSUM"))
    UN = {}
    YSB = {}
    pending = []

    def emit_outproj_unit():
        if not pending:
            return
        qcp, qi, ec = pending.pop(0)
        qt = qcp * 4 + qi
        if ec == 0:
            YSB[qt] = sb.tile([P, D], F32, tag="y", bufs=3, name=f"Y{qt}")
        ysb = YSB[qt]
        yps = ps2.tile([P, QC], F32, tag="st", bufs=2, name=f"yp{qt}_{ec}")
        for pr in range(2):
            nc.tensor.matmul(
                yps[:],
                (UN[qcp, pr][:, qi * P : (qi + 1) * P]),
                (woT[pr][:, ec * QC : (ec + 1) * QC]),
                start=(pr == 0),
                stop=(pr == 1),
            )
        nc.vector.tensor_copy(ysb[:, ec * QC : (ec + 1) * QC], yps[:])
        if ec == 1:
            nc.sync.dma_start(io["y"][qt * P : (qt + 1) * P, :], ysb[:])

    for qc in range(NQC):
        U = {
            h: ps2.tile([P, QC], F32, tag="u", bufs=4, name=f"U{qc}_{h}")
            for h in range(4)
        }
        for kg in range(NKT // 2):
            for h in range(4):
                pr, lo = h // 2, (h % 2) * 64
                st2 = ps2.tile([P, 2, QC], F32, tag="st", bufs=2, name=f"st{qc}_{kg}_{h}")
                for kk in range(2):
                    k_tile = kg * 2 + kk
                    sc, off = divmod(k_tile, 4)
                    nc.tensor.matmul(
                        st2[:, kk, :],
                        (KT[pr, sc][lo : lo + 64, off * P : (off + 1) * P]),
                        (QT[pr, qc][lo : lo + 64, :]),
                        start=True,
                        stop=True,
                        tile_position=(lo, 0),
                    )
                pt2 = sb.tile([P, 2, QC], BF16, tag="pt", bufs=3, name=f"pt{qc}_{kg}_{h}")
                nc.scalar.activation(pt2[:], st2[:], AF.Exp, scale=SCALE)
                for kk in range(2):
                    k_tile = kg * 2 + kk
                    nc.tensor.matmul(
                        U[h][0:65, :],
                        (V[k_tile][:, h, :]),
                        (pt2[:, kk, :]),
                        start=(kg == 0 and kk == 0),
                        stop=(kg == NKT // 2 - 1 and kk == 1),
                    )
            # one out-projection unit of the previous q-chunk per k-group:
            # independent PE work that fills the gaps while ACT runs exp,
            # keeping the PE busy enough that HAM stays at full clock.
            emit_outproj_unit()

        # normalize: UN_pair rows = U_h[0:64] * (1/Z_h); odd head's rows are
        # DMA-shifted into partitions 64..127 of the pair tile. The four
        # denominators are packed into partitions 0..3 of one tile so a single
        # RECIPROCAL covers all heads (DVE reciprocal is expensive per op).
        for pr in range(2):
            UN[qc, pr] = sb.tile([P, QC], BF16, tag="un", bufs=8, name=f"UN{qc}_{pr}")
        z4 = sb.tile([4, QC], F32, tag="z4", bufs=2, name=f"z4_{qc}")
        for h in range(4):
            zs = sb.tile([65, QC], F32, tag="zs", bufs=2, name=f"zs{qc}_{h}")
            nc.vector.tensor_copy(zs[64:65, :], U[h][64:65, :])
            nc.sync.dma_start(z4[h : h + 1, :], zs[64:65, :])
        rz4 = sb.tile([4, QC], F32, tag="rz4", bufs=2, name=f"rz4_{qc}")
        nc.vector.reciprocal(rz4[:], z4[:])
        for h in range(4):
            pr, off = h // 2, (h % 2) * 64
            if h == 0:
                r0 = rz4[0:1, :]
            else:
                r0t = sb.tile([1, QC], F32, tag="r0", bufs=3, name=f"r0_{qc}_{h}")
                nc.sync.dma_start(r0t[:], rz4[h : h + 1, :])
                r0 = r0t[:]
            rb = sb.tile([64, QC], F32, tag="rb", bufs=2, name=f"rb{qc}_{h}")
            nc.gpsimd.partition_broadcast(rb[:], r0, channels=64)
            if off == 0:
                nc.vector.tensor_mul(UN[qc, pr][0:64, :], U[h][0:64, :], rb[:])
            else:
                tmp = sb.tile([64, QC], BF16, tag="untmp", bufs=2, name=f"untmp{qc}_{h}")
                nc.vector.tensor_mul(tmp[:], U[h][0:64, :], rb[:])
                nc.sync.dma_start(UN[qc, pr][64:128, :], tmp[:])

        pending.extend((qc, qi, ec) for qi in range(4) for ec in range(2))

    while pending:
        emit_outproj_unit()


def build_program():
    nc = bacc.Bacc(
        "TRN2", target_bir_lowering=False, debug=False, num_devices=NCORES
    )
    io = {
        "xq": nc.dram_tensor("xq", [D, S], BF16, kind="ExternalInput").ap(),
        "xk": nc.dram_tensor("xk", [D, S], BF16, kind="ExternalInput").ap(),
        "xv": nc.dram_tensor("xv", [D, S], BF16, kind="ExternalInput").ap(),
        "wq": nc.dram_tensor("wq", [D, DG], BF16, kind="ExternalInput").ap(),
        "wk": nc.dram_tensor("wk", [D, DG], BF16, kind="ExternalInput").ap(),
        "wv": nc.dram_tensor("wv", [D, DG], BF16, kind="ExternalInput").ap(),
        "wo": nc.dram_tensor("wo", [DG, D], BF16, kind="ExternalInput").ap(),
        "bq": nc.dram_tensor("bq", [DG], F32, kind="ExternalInput").ap(),
        "bk": nc.dram_tensor("bk", [DG], F32, kind="ExternalInput").ap(),
        "bv": nc.dram_tensor("bv", [DG], BF16, kind="ExternalInput").ap(),
        "ones": nc.dram_tensor("ones", [P], BF16, kind="ExternalInput").ap(),
        "ones4": nc.dram_tensor("ones4", [P, 4], BF16, kind="ExternalInput").ap(),
        "y": nc.dram_tensor("y", [S, D], F32, kind="ExternalOutput").ap(),
    }
    with tile.TileContext(nc) as tc:
        with ExitStack() as ctx:
            _body(ctx, tc, io)
    nc.compile()
    return nc


_CACHE = {}


def _get_program():
    if "nc" not in _CACHE:
        _CACHE["nc"] = build_program()
    return _CACHE["nc"]


def make_in_maps(inputs):
    q = np.asarray(inputs["query"], np.float32)
    k = np.asarray(inputs["key"], np.float32)
    v = np.asarray(inputs["value"], np.float32)
    W_q = np.asarray(inputs["W_q"], np.float32)
    W_k = np.asarray(inputs["W_k"], np.float32)
    W_v = np.asarray(inputs["W_v"], np.float32)
    W_o = np.asarray(inputs["W_o"], np.float32)
    b_q = np.asarray(inputs["b_q"], np.float32)
    b_k = np.asarray(inputs["b_k"], np.float32)
    b_v = np.asarray(inputs["b_v"], np.float32)

    bf = ml_dtypes.bfloat16
    xT = [
        [np.ascontiguousarray(x[b].T).astype(bf) for b in range(B)]
        for x in (q, k, v)
    ]
    in_maps = []
    for core in range(NCORES):
        b, g = divmod(core, NG)
        sl = slice(g * DG, (g + 1) * DG)
        in_maps.append(
            {
                "xq": xT[0][b],
                "xk": xT[1][b],
                "xv": xT[2][b],
                "wq": np.ascontiguousarray(W_q[sl, :].T).astype(bf),
                "wk": np.ascontiguousarray(W_k[sl, :].T).astype(bf),
                "wv": np.ascontiguousarray(W_v[sl, :].T).astype(bf),
                "wo": np.ascontiguousarray(W_o[:, sl].T).astype(bf),
                "bq": np.ascontiguousarray(b_q[sl]),
                "bk": np.ascontiguousarray(b_k[sl]),
                "bv": np.ascontiguousarray(b_v[sl]).astype(bf),
                "ones": np.ones(P, bf),
                "ones4": np.ones((P, 4), bf),
            }
        )
    return in_maps


def kernel(**inputs):
    from concourse.bass_utils import run_bass_kernel_spmd

    nc = _get_program()
    in_maps = make_in_maps(inputs)
    trace = bool(int(os.environ.get("MHA_TRACE", "0")))
    res = run_bass_kernel_spmd(nc, in_maps, list(range(NCORES)), trace=trace)
    _CACHE["last_results"] = res

    b_o = np.asarray(inputs["b_o"], np.float32)
    out = np.zeros((B, S, D), np.float32)
    for core in range(NCORES):
        b = core // NG
        out[b] += res.results[core]["y"]
    out += b_o[None, None, :]
    return out
